# revision 29
# baseline (speedup 1.0000x reference)
"""MoE layer (router + 8 experts top-2 + shared expert) on 8 Trainium2 cores.

Strategy (expert-parallel, matching the all-to-all dispatch hint):
  - Host computes router logits/top-2/softmax and gathers each expert's
    tokens (the "all-to-all dispatch" — host-side since kernel() owns the
    full inputs and sharding).
  - Core c holds expert c's W1/W2 (bf16) and computes
        y_c = relu(x_gathered @ W1_c + b1_c) @ W2_c
    for its (padded-to-capacity) token set in transposed layout.
  - The always-on shared expert is d_ff-sliced 8 ways: core c computes
    partial_c = relu(x_all @ Ws1[:, c*512:(c+1)*512] + bs1) @ Ws2[slice]
    over all tokens; partials are summed on host.
  - Host applies gate weights, b2/bs2 biases, and scatter-adds expert
    outputs back to token order.

All matmuls run in bf16 (same PE rate as fp32r at N>=256, but half the
DMA traffic, FWL weight loads, and full-rate small-N tail chunks) with
fp32 PSUM accumulation.  The second FFN layer accumulates all 32 ff
k-tiles directly in PSUM (no vector read-modify-write of y).
"""

import os
import sys

import numpy as np

for _p in ("/opt/trn_rl_repo", os.path.expanduser("~/.axon_site/_ro/trn_rl_repo")):
    if os.path.isdir(_p) and _p not in sys.path:
        sys.path.append(_p)

import ml_dtypes  # noqa: E402

import concourse.bass as bass  # noqa: E402
import concourse.tile as tile  # noqa: E402
from concourse import bacc, mybir  # noqa: E402
from concourse.bass import ds, ts  # noqa: E402
from concourse.bass_utils import run_bass_kernel_spmd  # noqa: E402

D_MODEL, D_FF, N_EXP, TOP_K = 1024, 4096, 8, 2
P = 128
KD = D_MODEL // P        # 8 partition-tiles over d_model
MF = D_FF // P           # 32 partition-tiles over d_ff
FF_SH = D_FF // N_EXP    # 512: shared-expert d_ff slice per core
MS = FF_SH // P          # 4 partition-tiles over the shared slice
T_TOTAL = 4096
CH = 512                 # token-chunk width (one PSUM bank of fp32)

F32 = mybir.dt.float32
BF16 = mybir.dt.bfloat16
RELU = mybir.ActivationFunctionType.Relu

NPBF16 = ml_dtypes.bfloat16


def _chunks(total, step=CH, rem_first=True):
    """Split `total` into pieces <= step."""
    rem = total % step
    out = ([rem] if rem else []) + [step] * (total // step)
    if not rem_first and rem:
        out = out[1:] + [rem]
    offs = np.cumsum([0] + out[:-1]).tolist() if out else []
    return list(zip(offs, out))


def _declare_io(nc, C, timing=False):
    if timing:
        def inp(name, shape, dt):
            return nc.dram_tensor(name, shape, dt)
        out = inp
    else:
        def inp(name, shape, dt):
            return nc.declare_dram_parameter(name, shape, dt, isOutput=False)

        def out(name, shape, dt):
            return nc.declare_dram_parameter(name, shape, dt, isOutput=True)
    t = {}
    t["xg"] = inp("xg", [P, KD, C], BF16)
    t["xt"] = inp("xt", [P, KD, T_TOTAL], BF16)
    t["w1"] = inp("w1", [P, KD, D_FF], BF16)
    t["w2"] = inp("w2", [P, MF, D_MODEL], BF16)
    t["b1t"] = inp("b1t", [P, MF], F32)
    t["ws1"] = inp("ws1", [P, KD, FF_SH], BF16)
    t["ws2"] = inp("ws2", [P, MS, D_MODEL], BF16)
    t["bs1t"] = inp("bs1t", [P, MS], F32)
    t["yt"] = out("yt", [P, KD, C], BF16)
    t["st"] = out("st", [P, KD, T_TOTAL], BF16)
    if timing:
        # Tiny external output so the timing NEFF downloads ~nothing
        # (the real outputs above are Internal DRAM in timing mode).
        t["sink"] = nc.declare_dram_parameter("sink", [P, 4], BF16, isOutput=True)
    return t


def _emit_body(nc, tc, t, C, pre, no_dma=False):
    if no_dma:
        class _Skip:
            def dma_start(self, *a, **k):
                return None
        dma_q = _Skip()
        dma_w = _Skip()
    else:
        dma_q = nc.sync       # latency-critical stream: x chunks, outputs
        # w1/w2 bulk on the ACT queue: the interleaved activations pace the
        # triggers so the bulk can't flood the DMA engines ahead of the
        # x-token stream.  Shared-expert weights ride the otherwise-idle
        # GpSimd/SWDGE queue so the *next* loop iteration's phase A isn't
        # stuck behind this iteration's ACT work.
        dma_w = nc.scalar
        dma_s = nc.gpsimd
    xg, xt, w1, w2, b1t, ws1, ws2, bs1t, yt, st = (
        t["xg"], t["xt"], t["w1"], t["w2"], t["b1t"],
        t["ws1"], t["ws2"], t["bs1t"], t["yt"], t["st"],
    )
    # Phase-B chunks: three equal ~C/3 chunks (<=512 each) instead of
    # [512, 512, rem].  A narrow rem chunk (e.g. 68 cols) cannot hide its
    # per-matmul LDWEIGHTS (~53ns) under the column stream (~28ns), so the
    # tail would run weight-load-bound; equal chunks keep every matmul wide
    # enough (>=150ns) to cover the weight load.  Total stream cycles are
    # identical either way.
    if 3 * CH >= C >= 768:
        c1 = min(CH, -(-C // 3 + 3) // 4 * 4)
        c2 = min(CH, -(-(C - c1) // 2 + 3) // 4 * 4)
        sizes = [s for s in (c1, c2, C - c1 - c2) if s > 0]
        offs = np.cumsum([0] + sizes[:-1]).tolist()
        ccs = list(zip(offs, sizes))
    else:
        ccs = _chunks(C, rem_first=False)
    tcs = _chunks(T_TOTAL)

    w1_sb = pre["w1"]
    w2_sb = pre["w2"]
    ws1_sb = pre["ws1"]
    ws2_sb = pre["ws2"]
    b1_sb = pre["b1"]
    bs1_sb = pre["bs1"]
    with (
        tc.tile_pool(name="xs", bufs=2) as xs,
        tc.tile_pool(name="hp", bufs=1) as hp,
        tc.tile_pool(name="hsp", bufs=1) as hsp,
        tc.tile_pool(name="yp", bufs=1) as yp,
        tc.tile_pool(name="stp", bufs=1) as stp,
        tc.tile_pool(name="ph", bufs=3, space="PSUM") as ph,
        tc.tile_pool(name="py", bufs=3, space="PSUM") as py,
    ):

        # ---------------- Phase A: shared expert, d_ff slice, all tokens
        st_last = None
        for ci, (off, n) in enumerate(tcs):
            xt_c = xs.tile([P, KD, CH], BF16, tag="xs")
            dma_q.dma_start(out=xt_c[:, :, ds(0, n)], in_=xt[:, :, ds(off, n)])
            # Interleave slices of the expert-weight bulk chunk by chunk so
            # the x-token stream is never stuck behind a long weight queue;
            # half of w2 is deferred into phase B (needed only ~55us in).
            if ci == 0:
                dma_w.dma_start(out=b1_sb[:], in_=b1t[:])
            dma_w.dma_start(
                out=w1_sb[:, :, ds(ci * CH, CH)], in_=w1[:, :, ds(ci * CH, CH)]
            )
            dma_w.dma_start(
                out=w2_sb[:, ds(2 * ci, 2), :], in_=w2[:, ds(2 * ci, 2), :]
            )
            hs_sb = hsp.tile([P, MS, CH], BF16, tag="hs")
            for m in range(MS):
                ph_t = ph.tile([P, CH], F32, tag="ph")
                for k in range(KD):
                    nc.tensor.matmul(
                        ph_t[:, ds(0, n)],
                        ws1_sb[:, k, ts(m, P)],
                        xt_c[:, k, ds(0, n)],
                        start=(k == 0),
                        stop=(k == KD - 1),
                    )
                nc.scalar.activation(
                    out=hs_sb[:, m, ds(0, n)],
                    in_=ph_t[:, ds(0, n)],
                    func=RELU,
                    bias=bs1_sb[:, m : m + 1],
                )
            for jh in range(2):
                st_t = stp.tile([P, 4, CH], BF16, tag="st")
                for j4 in range(4):
                    j = jh * 4 + j4
                    py_t = py.tile([P, CH], F32, tag="py")
                    for m in range(MS):
                        nc.tensor.matmul(
                            py_t[:, ds(0, n)],
                            ws2_sb[:, m, ts(j, P)],
                            hs_sb[:, m, ds(0, n)],
                            start=(m == 0),
                            stop=(m == MS - 1),
                        )
                    nc.vector.tensor_copy(
                        out=st_t[:, j4, ds(0, n)], in_=py_t[:, ds(0, n)]
                    )
                dma_q.dma_start(
                    out=st[:, ds(jh * 4, 4), ds(off, n)], in_=st_t[:, :, ds(0, n)]
                )
                st_last = st_t

        # ---------------- Phase B: this core's expert on gathered tokens
        for bi, (off, n) in enumerate(ccs):
            xg_c = xs.tile([P, KD, CH], BF16, tag="xs")
            dma_q.dma_start(out=xg_c[:, :, ds(0, n)], in_=xg[:, :, ds(off, n)])
            h_sb = hp.tile([P, MF, CH], BF16, tag="h")
            for m in range(MF):
                if bi == 0 and m % 8 == 0:
                    # second half of w2, spread under phase-B chunk-0 L1
                    dma_w.dma_start(
                        out=w2_sb[:, ds(16 + m // 2, 4), :],
                        in_=w2[:, ds(16 + m // 2, 4), :],
                    )
                ph_t = ph.tile([P, CH], F32, tag="ph")
                for k in range(KD):
                    nc.tensor.matmul(
                        ph_t[:, ds(0, n)],
                        w1_sb[:, k, ts(m, P)],
                        xg_c[:, k, ds(0, n)],
                        start=(k == 0),
                        stop=(k == KD - 1),
                    )
                nc.scalar.activation(
                    out=h_sb[:, m, ds(0, n)],
                    in_=ph_t[:, ds(0, n)],
                    func=RELU,
                    bias=b1_sb[:, m : m + 1],
                )
            for jh in range(2):
                y_t = yp.tile([P, 4, CH], BF16, tag="y")
                for j4 in range(4):
                    j = jh * 4 + j4
                    py_t = py.tile([P, CH], F32, tag="py")
                    for m in range(MF):
                        nc.tensor.matmul(
                            py_t[:, ds(0, n)],
                            w2_sb[:, m, ts(j, P)],
                            h_sb[:, m, ds(0, n)],
                            start=(m == 0),
                            stop=(m == MF - 1),
                        )
                    nc.vector.tensor_copy(
                        out=y_t[:, j4, ds(0, n)], in_=py_t[:, ds(0, n)]
                    )
                dma_q.dma_start(
                    out=yt[:, ds(jh * 4, 4), ds(off, n)], in_=y_t[:, :, ds(0, n)]
                )

        # Prefetch the next iteration's phase-A criticals before the For_i
        # all-engine barrier: SBUF contents persist across the barrier, so
        # the next iteration's first matmuls have zero DMA latency.
        _prefetch_phase_a(nc, t, pre, no_dma)

        if "sink" in t and st_last is not None:
            nc.sync.dma_start(out=t["sink"][:], in_=st_last[:, 0, ds(0, 4)])


def _alloc_pre(tc, wk):
    specs = {
        "w1": ([P, KD, D_FF], BF16),
        "w2": ([P, MF, D_MODEL], BF16),
        "ws1": ([P, KD, FF_SH], BF16),
        "ws2": ([P, MS, D_MODEL], BF16),
        "b1": ([P, MF], F32),
        "bs1": ([P, MS], F32),
    }
    return {
        k: wk.tile(shape, dt, tag=k, name=f"pre_{k}")
        for k, (shape, dt) in specs.items()
    }


def _prefetch_phase_a(nc, t, pre, no_dma=False):
    if no_dma:
        return
    nc.gpsimd.dma_start(out=pre["bs1"][:], in_=t["bs1t"][:])
    nc.gpsimd.dma_start(out=pre["ws1"][:], in_=t["ws1"][:])
    nc.gpsimd.dma_start(out=pre["ws2"][:], in_=t["ws2"][:])


def build_program(C):
    nc = bacc.Bacc(None, target_bir_lowering=False, debug=False)
    t = _declare_io(nc, C, timing=False)
    with tile.TileContext(nc) as tc:
        with tc.tile_pool(name="wk", bufs=1) as wk:
            pre = _alloc_pre(tc, wk)
            _prefetch_phase_a(nc, t, pre)
            _emit_body(nc, tc, t, C, pre)
    nc.compile()
    return nc


def build_timing_program(C, trip, no_dma=False, unroll=1):
    """Timing variant: inputs/outputs are Internal DRAM (no host transfer
    except a tiny sink), body repeated `trip` times in a hardware loop.
    The body is unrolled `unroll`x inside the loop so the For_i all-engine
    barrier and head-prefetch latency amortize across bodies."""
    assert trip % unroll == 0
    nc = bacc.Bacc(None, target_bir_lowering=False, debug=False)
    t = _declare_io(nc, C, timing=True)
    with tile.TileContext(nc) as tc:
        with tc.tile_pool(name="wk", bufs=1) as wk:
            pre = _alloc_pre(tc, wk)
            _prefetch_phase_a(nc, t, pre, no_dma)
            with tc.For_i(0, trip // unroll, 1):
                for _ in range(unroll):
                    _emit_body(nc, tc, t, C, pre, no_dma=no_dma)
    nc.compile()
    return nc


def _to_tiles(a2d, dt=NPBF16):
    """[R, N] with R = r_tiles*128 -> [128, r_tiles, N] so element
    [p, r, n] = a2d[r*128 + p, n]; contiguous for a single straight DMA."""
    R, N = a2d.shape
    return np.ascontiguousarray(
        a2d.reshape(R // P, P, N).transpose(1, 0, 2).astype(dt)
    )


def _from_tiles(a3d):
    """Inverse of _to_tiles: [128, r_tiles, N] -> [r_tiles*128, N]."""
    p, r, n = a3d.shape
    return a3d.astype(np.float32).transpose(1, 0, 2).reshape(r * p, n)


def _route(xf, Wg):
    """Replicates TopKRouter eval: top-2 by logit, softmax over the two."""
    logits = xf @ Wg
    top_idx = np.argsort(-logits, axis=1, kind="stable")[:, :TOP_K]
    top_vals = np.take_along_axis(logits, top_idx, axis=1)
    e = np.exp(top_vals - top_vals.max(axis=1, keepdims=True))
    top_w = (e / e.sum(axis=1, keepdims=True)).astype(np.float32)
    return top_idx, top_w


_PROG_CACHE = {}


def _get_program(C):
    if C not in _PROG_CACHE:
        _PROG_CACHE[C] = build_program(C)
    return _PROG_CACHE[C]


def make_in_maps(x, Wg, W1, b1, W2, b2, Ws1, bs1, Ws2, bs2):
    """Host-side routing + sharding. Returns (in_maps, C, idx_e, gate_e, xf)."""
    B, S, D = x.shape
    T = B * S
    xf = np.ascontiguousarray(np.asarray(x, np.float32).reshape(T, D))
    top_idx, top_w = _route(xf, np.asarray(Wg, np.float32))

    idx_e, gate_e = [], []
    for ex in range(N_EXP):
        rows, slot = np.nonzero(top_idx == ex)
        idx_e.append(rows)
        gate_e.append(top_w[rows, slot])
    counts = [len(i) for i in idx_e]
    C = max(4, -(-max(counts) // 4) * 4)

    xt_tiled = _to_tiles(xf.T)  # [128, 8, 4096] bf16
    in_maps = []
    for ex in range(N_EXP):
        xg = np.zeros((C, D_MODEL), np.float32)
        xg[: counts[ex]] = xf[idx_e[ex]]
        sl = slice(ex * FF_SH, (ex + 1) * FF_SH)
        in_maps.append(
            {
                "xg": _to_tiles(np.ascontiguousarray(xg.T)),
                "xt": xt_tiled,
                "w1": _to_tiles(np.asarray(W1[ex], np.float32)),
                "w2": _to_tiles(np.asarray(W2[ex], np.float32)),
                "b1t": np.ascontiguousarray(
                    np.asarray(b1[ex], np.float32).reshape(MF, P).T
                ),
                "ws1": _to_tiles(np.asarray(Ws1[:, sl], np.float32)),
                "ws2": _to_tiles(np.asarray(Ws2[sl, :], np.float32)),
                "bs1t": np.ascontiguousarray(
                    np.asarray(bs1[sl], np.float32).reshape(MS, P).T
                ),
            }
        )
    return in_maps, C, idx_e, gate_e, xf


def assemble_output(results, shape, C, idx_e, gate_e, b2, bs2):
    B, S, D = shape
    T = B * S
    out = np.zeros((T, D), np.float32)
    for ex in range(N_EXP):
        out += _from_tiles(results[ex]["st"]).T  # shared partials
    out += np.asarray(bs2, np.float32)[None, :]
    b2 = np.asarray(b2, np.float32)
    for ex in range(N_EXP):
        y = _from_tiles(results[ex]["yt"]).T[: len(idx_e[ex])]
        out[idx_e[ex]] += gate_e[ex][:, None] * (y + b2[ex][None, :])
    return out.reshape(B, S, D)


def kernel(x, Wg, W1, b1, W2, b2, Ws1, bs1, Ws2, bs2):
    in_maps, C, idx_e, gate_e, _ = make_in_maps(
        x, Wg, W1, b1, W2, b2, Ws1, bs1, Ws2, bs2
    )
    nc = _get_program(C)
    res = run_bass_kernel_spmd(nc, in_maps, list(range(N_EXP)))
    return assemble_output(
        res.results, x.shape, C, idx_e, gate_e, b2, bs2
    ).astype(np.float32)


# revision 30
# speedup vs baseline: 1.0813x; 1.0813x over previous
"""MoE layer (router + 8 experts top-2 + shared expert) on 8 Trainium2 cores.

Strategy (expert-parallel, matching the all-to-all dispatch hint):
  - Host computes router logits/top-2/softmax and gathers each expert's
    tokens (the "all-to-all dispatch" — host-side since kernel() owns the
    full inputs and sharding).
  - Core c holds expert c's W1/W2 (bf16) and computes
        y_c = relu(x_gathered @ W1_c + b1_c) @ W2_c
    for its (padded-to-capacity) token set in transposed layout.
  - The always-on shared expert is d_ff-sliced 8 ways: core c computes
    partial_c = relu(x_all @ Ws1[:, c*512:(c+1)*512] + bs1) @ Ws2[slice]
    over all tokens; partials are summed on host.
  - Host applies gate weights, b2/bs2 biases, and scatter-adds expert
    outputs back to token order.

All matmuls run in bf16 (same PE rate as fp32r at N>=256, but half the
DMA traffic, FWL weight loads, and full-rate small-N tail chunks) with
fp32 PSUM accumulation.  The second FFN layer accumulates all 32 ff
k-tiles directly in PSUM (no vector read-modify-write of y).
"""

import os
import sys

import numpy as np

for _p in ("/opt/trn_rl_repo", os.path.expanduser("~/.axon_site/_ro/trn_rl_repo")):
    if os.path.isdir(_p) and _p not in sys.path:
        sys.path.append(_p)

import ml_dtypes  # noqa: E402

import concourse.bass as bass  # noqa: E402
import concourse.tile as tile  # noqa: E402
from concourse import bacc, mybir  # noqa: E402
from concourse.bass import ds, ts  # noqa: E402
from concourse.bass_utils import run_bass_kernel_spmd  # noqa: E402

D_MODEL, D_FF, N_EXP, TOP_K = 1024, 4096, 8, 2
P = 128
KD = D_MODEL // P        # 8 partition-tiles over d_model
MF = D_FF // P           # 32 partition-tiles over d_ff
FF_SH = D_FF // N_EXP    # 512: shared-expert d_ff slice per core
MS = FF_SH // P          # 4 partition-tiles over the shared slice
T_TOTAL = 4096
CH = 512                 # token-chunk width (one PSUM bank of fp32)

F32 = mybir.dt.float32
BF16 = mybir.dt.bfloat16
RELU = mybir.ActivationFunctionType.Relu

NPBF16 = ml_dtypes.bfloat16


def _chunks(total, step=CH, rem_first=True):
    """Split `total` into pieces <= step."""
    rem = total % step
    out = ([rem] if rem else []) + [step] * (total // step)
    if not rem_first and rem:
        out = out[1:] + [rem]
    offs = np.cumsum([0] + out[:-1]).tolist() if out else []
    return list(zip(offs, out))


def _declare_io(nc, C, timing=False):
    if timing:
        def inp(name, shape, dt):
            return nc.dram_tensor(name, shape, dt)
        out = inp
    else:
        def inp(name, shape, dt):
            return nc.declare_dram_parameter(name, shape, dt, isOutput=False)

        def out(name, shape, dt):
            return nc.declare_dram_parameter(name, shape, dt, isOutput=True)
    t = {}
    t["xg"] = inp("xg", [P, KD, C], BF16)
    t["xt"] = inp("xt", [P, KD, T_TOTAL], BF16)
    t["w1"] = inp("w1", [P, KD, D_FF], BF16)
    t["w2"] = inp("w2", [P, MF, D_MODEL], BF16)
    t["b1t"] = inp("b1t", [P, MF], F32)
    t["ws1"] = inp("ws1", [P, KD, FF_SH], BF16)
    t["ws2"] = inp("ws2", [P, MS, D_MODEL], BF16)
    t["bs1t"] = inp("bs1t", [P, MS], F32)
    t["yt"] = out("yt", [P, KD, C], BF16)
    t["st"] = out("st", [P, KD, T_TOTAL], BF16)
    if timing:
        # Tiny external output so the timing NEFF downloads ~nothing
        # (the real outputs above are Internal DRAM in timing mode).
        t["sink"] = nc.declare_dram_parameter("sink", [P, 4], BF16, isOutput=True)
    return t


def _emit_body(nc, tc, t, C, pre, no_dma=False):
    if no_dma:
        class _Skip:
            def dma_start(self, *a, **k):
                return None
        dma_q = _Skip()
        dma_w = _Skip()
    else:
        dma_q = nc.sync       # latency-critical stream: x chunks, outputs
        # w1/w2 bulk on the ACT queue: the interleaved activations pace the
        # triggers so the bulk can't flood the DMA engines ahead of the
        # x-token stream.  Shared-expert weights ride the otherwise-idle
        # GpSimd/SWDGE queue so the *next* loop iteration's phase A isn't
        # stuck behind this iteration's ACT work.
        dma_w = nc.scalar
        dma_s = nc.gpsimd
    xg, xt, w1, w2, b1t, ws1, ws2, bs1t, yt, st = (
        t["xg"], t["xt"], t["w1"], t["w2"], t["b1t"],
        t["ws1"], t["ws2"], t["bs1t"], t["yt"], t["st"],
    )
    # Phase-B chunks: three equal ~C/3 chunks (<=512 each) instead of
    # [512, 512, rem].  A narrow rem chunk (e.g. 68 cols) cannot hide its
    # per-matmul LDWEIGHTS (~53ns) under the column stream (~28ns), so the
    # tail would run weight-load-bound; equal chunks keep every matmul wide
    # enough (>=150ns) to cover the weight load.  Total stream cycles are
    # identical either way.
    if 1500 >= C >= 768:
        c1 = min(CH, -(-C // 3 + 3) // 4 * 4)
        c2 = min(CH, -(-(C - c1) // 2 + 3) // 4 * 4)
        sizes = [s for s in (c1, c2, C - c1 - c2) if s > 0]
        offs = np.cumsum([0] + sizes[:-1]).tolist()
        ccs = list(zip(offs, sizes))
    else:
        ccs = _chunks(C, rem_first=False)
    tcs = _chunks(T_TOTAL)

    w1_sb = pre["w1"]
    w2_sb = pre["w2"]
    ws1_sb = pre["ws1"]
    ws2_sb = pre["ws2"]
    b1_sb = pre["b1"]
    bs1_sb = pre["bs1"]
    with (
        tc.tile_pool(name="xs", bufs=2) as xs,
        tc.tile_pool(name="hp", bufs=1) as hp,
        tc.tile_pool(name="hsp", bufs=1) as hsp,
        tc.tile_pool(name="yp", bufs=1) as yp,
        tc.tile_pool(name="stp", bufs=1) as stp,
        tc.tile_pool(name="ph", bufs=3, space="PSUM") as ph,
        tc.tile_pool(name="py", bufs=3, space="PSUM") as py,
    ):

        # ---------------- Phase A: shared expert, d_ff slice, all tokens
        st_last = None
        for ci, (off, n) in enumerate(tcs):
            xt_c = xs.tile([P, KD, CH], BF16, tag="xs")
            dma_q.dma_start(out=xt_c[:, :, ds(0, n)], in_=xt[:, :, ds(off, n)])
            # Interleave slices of the expert-weight bulk chunk by chunk so
            # the x-token stream is never stuck behind a long weight queue;
            # half of w2 is deferred into phase B (needed only ~55us in).
            if ci == 0:
                dma_w.dma_start(out=b1_sb[:], in_=b1t[:])
            dma_w.dma_start(
                out=w1_sb[:, :, ds(ci * CH, CH)], in_=w1[:, :, ds(ci * CH, CH)]
            )
            dma_w.dma_start(
                out=w2_sb[:, ds(2 * ci, 2), :], in_=w2[:, ds(2 * ci, 2), :]
            )
            hs_sb = hsp.tile([P, MS, CH], BF16, tag="hs")
            for m in range(MS):
                ph_t = ph.tile([P, CH], F32, tag="ph")
                for k in range(KD):
                    nc.tensor.matmul(
                        ph_t[:, ds(0, n)],
                        ws1_sb[:, k, ts(m, P)],
                        xt_c[:, k, ds(0, n)],
                        start=(k == 0),
                        stop=(k == KD - 1),
                    )
                nc.scalar.activation(
                    out=hs_sb[:, m, ds(0, n)],
                    in_=ph_t[:, ds(0, n)],
                    func=RELU,
                    bias=bs1_sb[:, m : m + 1],
                )
            for jh in range(2):
                st_t = stp.tile([P, 4, CH], BF16, tag="st")
                for j4 in range(4):
                    j = jh * 4 + j4
                    py_t = py.tile([P, CH], F32, tag="py")
                    for m in range(MS):
                        nc.tensor.matmul(
                            py_t[:, ds(0, n)],
                            ws2_sb[:, m, ts(j, P)],
                            hs_sb[:, m, ds(0, n)],
                            start=(m == 0),
                            stop=(m == MS - 1),
                        )
                    nc.vector.tensor_copy(
                        out=st_t[:, j4, ds(0, n)], in_=py_t[:, ds(0, n)]
                    )
                dma_q.dma_start(
                    out=st[:, ds(jh * 4, 4), ds(off, n)], in_=st_t[:, :, ds(0, n)]
                )
                st_last = st_t

        # ---------------- Phase B: this core's expert on gathered tokens
        for bi, (off, n) in enumerate(ccs):
            xg_c = xs.tile([P, KD, CH], BF16, tag="xs")
            dma_q.dma_start(out=xg_c[:, :, ds(0, n)], in_=xg[:, :, ds(off, n)])
            h_sb = hp.tile([P, MF, CH], BF16, tag="h")
            for m in range(MF):
                if bi == 0 and m % 8 == 0:
                    # second half of w2, spread under phase-B chunk-0 L1
                    dma_w.dma_start(
                        out=w2_sb[:, ds(16 + m // 2, 4), :],
                        in_=w2[:, ds(16 + m // 2, 4), :],
                    )
                ph_t = ph.tile([P, CH], F32, tag="ph")
                for k in range(KD):
                    nc.tensor.matmul(
                        ph_t[:, ds(0, n)],
                        w1_sb[:, k, ts(m, P)],
                        xg_c[:, k, ds(0, n)],
                        start=(k == 0),
                        stop=(k == KD - 1),
                    )
                nc.scalar.activation(
                    out=h_sb[:, m, ds(0, n)],
                    in_=ph_t[:, ds(0, n)],
                    func=RELU,
                    bias=b1_sb[:, m : m + 1],
                )
            for jh in range(2):
                y_t = yp.tile([P, 4, CH], BF16, tag="y")
                for j4 in range(4):
                    j = jh * 4 + j4
                    py_t = py.tile([P, CH], F32, tag="py")
                    for m in range(MF):
                        nc.tensor.matmul(
                            py_t[:, ds(0, n)],
                            w2_sb[:, m, ts(j, P)],
                            h_sb[:, m, ds(0, n)],
                            start=(m == 0),
                            stop=(m == MF - 1),
                        )
                    nc.vector.tensor_copy(
                        out=y_t[:, j4, ds(0, n)], in_=py_t[:, ds(0, n)]
                    )
                dma_q.dma_start(
                    out=yt[:, ds(jh * 4, 4), ds(off, n)], in_=y_t[:, :, ds(0, n)]
                )

        # Prefetch the next iteration's phase-A criticals before the For_i
        # all-engine barrier: SBUF contents persist across the barrier, so
        # the next iteration's first matmuls have zero DMA latency.
        _prefetch_phase_a(nc, t, pre, no_dma)

        if "sink" in t and st_last is not None:
            nc.sync.dma_start(out=t["sink"][:], in_=st_last[:, 0, ds(0, 4)])


def _alloc_pre(tc, wk):
    specs = {
        "w1": ([P, KD, D_FF], BF16),
        "w2": ([P, MF, D_MODEL], BF16),
        "ws1": ([P, KD, FF_SH], BF16),
        "ws2": ([P, MS, D_MODEL], BF16),
        "b1": ([P, MF], F32),
        "bs1": ([P, MS], F32),
    }
    return {
        k: wk.tile(shape, dt, tag=k, name=f"pre_{k}")
        for k, (shape, dt) in specs.items()
    }


def _prefetch_phase_a(nc, t, pre, no_dma=False):
    if no_dma:
        return
    nc.gpsimd.dma_start(out=pre["bs1"][:], in_=t["bs1t"][:])
    nc.gpsimd.dma_start(out=pre["ws1"][:], in_=t["ws1"][:])
    nc.gpsimd.dma_start(out=pre["ws2"][:], in_=t["ws2"][:])


def build_program(C):
    nc = bacc.Bacc(None, target_bir_lowering=False, debug=False)
    t = _declare_io(nc, C, timing=False)
    with tile.TileContext(nc) as tc:
        with tc.tile_pool(name="wk", bufs=1) as wk:
            pre = _alloc_pre(tc, wk)
            _prefetch_phase_a(nc, t, pre)
            _emit_body(nc, tc, t, C, pre)
    nc.compile()
    return nc


def build_timing_program(C, trip, no_dma=False, unroll=1):
    """Timing variant: inputs/outputs are Internal DRAM (no host transfer
    except a tiny sink), body repeated `trip` times in a hardware loop.
    The body is unrolled `unroll`x inside the loop so the For_i all-engine
    barrier and head-prefetch latency amortize across bodies."""
    assert trip % unroll == 0
    nc = bacc.Bacc(None, target_bir_lowering=False, debug=False)
    t = _declare_io(nc, C, timing=True)
    with tile.TileContext(nc) as tc:
        with tc.tile_pool(name="wk", bufs=1) as wk:
            pre = _alloc_pre(tc, wk)
            _prefetch_phase_a(nc, t, pre, no_dma)
            with tc.For_i(0, trip // unroll, 1):
                for _ in range(unroll):
                    _emit_body(nc, tc, t, C, pre, no_dma=no_dma)
    nc.compile()
    return nc


def _to_tiles(a2d, dt=NPBF16):
    """[R, N] with R = r_tiles*128 -> [128, r_tiles, N] so element
    [p, r, n] = a2d[r*128 + p, n]; contiguous for a single straight DMA."""
    R, N = a2d.shape
    return np.ascontiguousarray(
        a2d.reshape(R // P, P, N).transpose(1, 0, 2).astype(dt)
    )


def _from_tiles(a3d):
    """Inverse of _to_tiles: [128, r_tiles, N] -> [r_tiles*128, N]."""
    p, r, n = a3d.shape
    return a3d.astype(np.float32).transpose(1, 0, 2).reshape(r * p, n)


def _route(xf, Wg):
    """Replicates TopKRouter eval: top-2 by logit, softmax over the two."""
    logits = xf @ Wg
    top_idx = np.argsort(-logits, axis=1, kind="stable")[:, :TOP_K]
    top_vals = np.take_along_axis(logits, top_idx, axis=1)
    e = np.exp(top_vals - top_vals.max(axis=1, keepdims=True))
    top_w = (e / e.sum(axis=1, keepdims=True)).astype(np.float32)
    return top_idx, top_w


_PROG_CACHE = {}


def _get_program(C):
    if C not in _PROG_CACHE:
        _PROG_CACHE[C] = build_program(C)
    return _PROG_CACHE[C]


def make_in_maps(x, Wg, W1, b1, W2, b2, Ws1, bs1, Ws2, bs2):
    """Host-side routing + sharding. Returns (in_maps, C, idx_e, gate_e, xf)."""
    B, S, D = x.shape
    T = B * S
    xf = np.ascontiguousarray(np.asarray(x, np.float32).reshape(T, D))
    top_idx, top_w = _route(xf, np.asarray(Wg, np.float32))

    idx_e, gate_e = [], []
    for ex in range(N_EXP):
        rows, slot = np.nonzero(top_idx == ex)
        idx_e.append(rows)
        gate_e.append(top_w[rows, slot])
    counts = [len(i) for i in idx_e]
    C = max(4, -(-max(counts) // 4) * 4)

    xt_tiled = _to_tiles(xf.T)  # [128, 8, 4096] bf16
    in_maps = []
    for ex in range(N_EXP):
        xg = np.zeros((C, D_MODEL), np.float32)
        xg[: counts[ex]] = xf[idx_e[ex]]
        sl = slice(ex * FF_SH, (ex + 1) * FF_SH)
        in_maps.append(
            {
                "xg": _to_tiles(np.ascontiguousarray(xg.T)),
                "xt": xt_tiled,
                "w1": _to_tiles(np.asarray(W1[ex], np.float32)),
                "w2": _to_tiles(np.asarray(W2[ex], np.float32)),
                "b1t": np.ascontiguousarray(
                    np.asarray(b1[ex], np.float32).reshape(MF, P).T
                ),
                "ws1": _to_tiles(np.asarray(Ws1[:, sl], np.float32)),
                "ws2": _to_tiles(np.asarray(Ws2[sl, :], np.float32)),
                "bs1t": np.ascontiguousarray(
                    np.asarray(bs1[sl], np.float32).reshape(MS, P).T
                ),
            }
        )
    return in_maps, C, idx_e, gate_e, xf


def assemble_output(results, shape, C, idx_e, gate_e, b2, bs2):
    B, S, D = shape
    T = B * S
    out = np.zeros((T, D), np.float32)
    for ex in range(N_EXP):
        out += _from_tiles(results[ex]["st"]).T  # shared partials
    out += np.asarray(bs2, np.float32)[None, :]
    b2 = np.asarray(b2, np.float32)
    for ex in range(N_EXP):
        y = _from_tiles(results[ex]["yt"]).T[: len(idx_e[ex])]
        out[idx_e[ex]] += gate_e[ex][:, None] * (y + b2[ex][None, :])
    return out.reshape(B, S, D)


def kernel(x, Wg, W1, b1, W2, b2, Ws1, bs1, Ws2, bs2):
    in_maps, C, idx_e, gate_e, _ = make_in_maps(
        x, Wg, W1, b1, W2, b2, Ws1, bs1, Ws2, bs2
    )
    nc = _get_program(C)
    res = run_bass_kernel_spmd(nc, in_maps, list(range(N_EXP)))
    return assemble_output(
        res.results, x.shape, C, idx_e, gate_e, b2, bs2
    ).astype(np.float32)
